# revision 18
# baseline (speedup 1.0000x reference)
"""Multi-head attention (per-head full-dim projections) on 8 TRN2 NeuronCores.

Problem: B=16, N=1024, D=512, H=8
  k_/v_/q_ = x @ W{k,v,q}[h].T + b  -> per-head [B,N,D]
  attn = softmax((q_ @ k_^T)/sqrt(D)); out = attn @ v_
  rep = interleave_heads(out) @ Wo.T + bo

Sharding: data parallel over batch (2 batches/core, no collectives).

Device kernel math notes (per core, per batch b, head h):
  - All activations kept "T-layout" [feature, token] so every matmul contracts
    over the partition dim with no on-device transposes:
      qhT[e,i] = WqT[h][d,e].T-mm  (lhsT=WqT tile, rhs=qT)  (+bq, x D^-0.5)
      khT[e,j] likewise (bias bk dropped: constant-per-row shift cancels in
      softmax), vh[j,e] = (lhsT=vT tile, rhs=WvT)  (bias bv folded into cv)
      S_T[j,i] = (lhsT=khT tile, rhs=qhT);  E_T = exp(S_T)  (no max-subtract:
      scores ~ N(0,1), max |s| ~ 6, exp is safe in fp32/bf16)
      den[1,i] = ones.T @ E_T;  recip = 1/den  (broadcast via DRAM roundtrip)
      numT -> oh[i,e]: actually num[i? no:] numT computed as
      rep[i,o] += (lhsT=ohT tile, rhs=WoTh[h]) accumulated over heads in SBUF
  - cv[o] = bo[o] + sum_h sum_e bv[h,e]*Wo[o, e*H+h] precomputed on host and
    added once (v-bias shifts oh by a constant vector -> constant rep shift).
"""

import sys

sys.path.insert(0, "/opt/trn_rl_repo")

from contextlib import ExitStack

import numpy as np
import ml_dtypes

B, N, D, H = 16, 1024, 512, 8
NCORES = 8
BPC = B // NCORES  # batches per core
P = 128
DC = D // P        # 4 feature chunks
NT = N // P        # 8 token chunks
FD = 512           # matmul moving free dim / PSUM bank
IH = N // FD       # 2 halves of the token axis

BF16 = ml_dtypes.bfloat16

_cached = {}


def _build():
    import concourse.bass as bass
    import concourse.tile as tile
    from concourse import bacc, mybir

    f32 = mybir.dt.float32
    bf16 = mybir.dt.bfloat16

    nc = bacc.Bacc(None, target_bir_lowering=False, debug=False)

    qT_d = nc.dram_tensor("qT", [BPC, D, N], bf16, kind="ExternalInput")
    kT_d = nc.dram_tensor("kT", [BPC, D, N], bf16, kind="ExternalInput")
    vT_d = nc.dram_tensor("vT", [BPC, D, N], bf16, kind="ExternalInput")
    wq_d = nc.dram_tensor("WqT", [H, D, D], bf16, kind="ExternalInput")
    wk_d = nc.dram_tensor("WkT", [H, D, D], bf16, kind="ExternalInput")
    wv_d = nc.dram_tensor("WvT", [H, D, D], bf16, kind="ExternalInput")
    wo_d = nc.dram_tensor("WoTh", [H, D, D], bf16, kind="ExternalInput")
    bq_d = nc.dram_tensor("bqp", [H, P, DC], f32, kind="ExternalInput")
    cv_d = nc.dram_tensor("cv", [1, D], f32, kind="ExternalInput")
    out_d = nc.dram_tensor("out", [BPC, N, D], f32, kind="ExternalOutput")

    scale = float(D) ** -0.5

    with tile.TileContext(nc) as tc, ExitStack() as ctx:
        consts = ctx.enter_context(tc.tile_pool(name="consts", bufs=1))
        acts = ctx.enter_context(tc.tile_pool(name="acts", bufs=1))
        wpool = ctx.enter_context(tc.tile_pool(name="wpool", bufs=2))
        projp = ctx.enter_context(tc.tile_pool(name="projp", bufs=2))
        etp = ctx.enter_context(tc.tile_pool(name="etp", bufs=1))
        ohp = ctx.enter_context(tc.tile_pool(name="ohp", bufs=2))
        rcp = ctx.enter_context(tc.tile_pool(name="rcp", bufs=2))
        repp = ctx.enter_context(tc.tile_pool(name="repp", bufs=2))
        mmps = ctx.enter_context(tc.tile_pool(name="mmps", bufs=4, space="PSUM"))
        repps = ctx.enter_context(tc.tile_pool(name="repps", bufs=4, space="PSUM"))
        dramp = ctx.enter_context(tc.tile_pool(name="dramp", bufs=2, space="DRAM"))

        bq_sb = consts.tile([P, H, DC], f32)
        nc.sync.dma_start(out=bq_sb[:], in_=bq_d[:].rearrange("h p c -> p h c"))
        cv_full = consts.tile([P, D], f32)
        nc.sync.dma_start(out=cv_full[:], in_=cv_d[0:1, :].to_broadcast([P, D]))

        for b in range(BPC):
            def load_weights(h):
                wq = wpool.tile([P, DC, D], bf16, name="wq")
                wk = wpool.tile([P, DC, D], bf16, name="wk")
                wv = wpool.tile([P, DC, D], bf16, name="wv")
                wo = wpool.tile([P, DC, D], bf16, name="wo")
                nc.sync.dma_start(out=wq[:], in_=wq_d[h].rearrange("(c p) e -> p c e", p=P))
                nc.sync.dma_start(out=wk[:], in_=wk_d[h].rearrange("(c p) e -> p c e", p=P))
                nc.sync.dma_start(out=wv[:], in_=wv_d[h].rearrange("(c p) e -> p c e", p=P))
                nc.sync.dma_start(out=wo[:], in_=wo_d[h].rearrange("(c p) e -> p c e", p=P))
                return wq, wk, wv, wo

            qT = acts.tile([P, DC, N], bf16, name="qT_sb")
            kT = acts.tile([P, DC, N], bf16, name="kT_sb")
            vT = acts.tile([P, DC, N], bf16, name="vT_sb")
            # interleave weight-h0 and activation chunk DMAs so the first
            # projection matmuls (wq dc0 + qT dc0) can start ASAP
            w0 = None
            for dc in range(DC):
                if b == 0 and dc == 0:
                    nc.sync.dma_start(out=qT[:, 0, :], in_=qT_d[b, 0:P, :])
                    w0 = load_weights(0)
                    nc.sync.dma_start(out=kT[:, 0, :], in_=kT_d[b, 0:P, :])
                    nc.sync.dma_start(out=vT[:, 0, :], in_=vT_d[b, 0:P, :])
                    continue
                nc.sync.dma_start(out=qT[:, dc, :], in_=qT_d[b, dc * P:(dc + 1) * P, :])
                nc.sync.dma_start(out=kT[:, dc, :], in_=kT_d[b, dc * P:(dc + 1) * P, :])
                nc.sync.dma_start(out=vT[:, dc, :], in_=vT_d[b, dc * P:(dc + 1) * P, :])

            rep = repp.tile([P, NT, D], f32, name="rep")

            for h in range(H):
                if b == 0 and h == 0:
                    wq, wk, wv, wo = w0
                else:
                    wq, wk, wv, wo = load_weights(h)

                qhT = projp.tile([P, DC, N], bf16, name="qhT")  # [e, i]
                khT = projp.tile([P, DC, N], bf16, name="khT")  # [e, j]
                vh = projp.tile([P, NT, D], bf16, name="vh")    # [j, e]

                # ---- projections ----
                for ec in range(DC):
                    pq = [mmps.tile([P, FD], f32, name="mm") for _ in range(IH)]
                    for dc in range(DC):
                        for ih in range(IH):
                            nc.tensor.matmul(
                                pq[ih][:],
                                lhsT=wq[:, dc, ec * P:(ec + 1) * P],
                                rhs=qT[:, dc, ih * FD:(ih + 1) * FD],
                                start=(dc == 0),
                                stop=(dc == DC - 1),
                            )
                    for ih in range(IH):
                        # qhT = psum*scale + bq*scale (bqp pre-scaled on host)
                        nc.scalar.activation(
                            out=qhT[:, ec, ih * FD:(ih + 1) * FD],
                            in_=pq[ih][:],
                            func=mybir.ActivationFunctionType.Identity,
                            bias=bq_sb[:, h, ec:ec + 1],
                            scale=scale,
                        )
                for ec in range(DC):
                    pk = [mmps.tile([P, FD], f32, name="mm") for _ in range(IH)]
                    for dc in range(DC):
                        for ih in range(IH):
                            nc.tensor.matmul(
                                pk[ih][:],
                                lhsT=wk[:, dc, ec * P:(ec + 1) * P],
                                rhs=kT[:, dc, ih * FD:(ih + 1) * FD],
                                start=(dc == 0),
                                stop=(dc == DC - 1),
                            )
                    for ih in range(IH):
                        nc.scalar.copy(out=khT[:, ec, ih * FD:(ih + 1) * FD], in_=pk[ih][:])
                for jc in range(NT):
                    pv = mmps.tile([P, FD], f32, name="mm")
                    for dc in range(DC):
                        nc.tensor.matmul(
                            pv[:],
                            lhsT=vT[:, dc, jc * P:(jc + 1) * P],
                            rhs=wv[:, dc, :],
                            start=(dc == 0),
                            stop=(dc == DC - 1),
                        )
                    nc.scalar.copy(out=vh[:, jc, :], in_=pv[:])

                # ---- scores + exp; den accumulated incrementally on DVE ----
                et = etp.tile([P, NT, N], bf16, name="et")  # E_T [j, i]
                den_acc = rcp.tile([P, N], f32, name="den_acc")
                for jc in range(NT):
                    ps = [mmps.tile([P, FD], f32, name="mm") for _ in range(IH)]
                    for ec in range(DC):
                        for ih in range(IH):
                            nc.tensor.matmul(
                                ps[ih][:],
                                lhsT=khT[:, ec, jc * P:(jc + 1) * P],
                                rhs=qhT[:, ec, ih * FD:(ih + 1) * FD],
                                start=(ec == 0),
                                stop=(ec == DC - 1),
                            )
                    for ih in range(IH):
                        nc.scalar.activation(
                            out=et[:, jc, ih * FD:(ih + 1) * FD],
                            in_=ps[ih][:],
                            func=mybir.ActivationFunctionType.Exp,
                        )
                    if jc == 1:
                        nc.vector.tensor_add(den_acc[:], et[:, 0, :], et[:, 1, :])
                    elif jc > 1:
                        nc.vector.tensor_add(den_acc[:], et[:, jc, :], den_acc[:])

                # ---- partition-reduce den on gpsimd, recip -> [i%P, i//P] ----
                from concourse import bass_isa

                nc.gpsimd.partition_all_reduce(
                    den_acc[:], den_acc[:], P, bass_isa.ReduceOp.add
                )
                rscratch = dramp.tile([1, N], f32, name="rscratch")
                nc.sync.dma_start(out=rscratch[:], in_=den_acc[0:1, :])
                den_pp = rcp.tile([P, NT], f32, name="den_pp")
                nc.sync.dma_start(
                    out=den_pp[:],
                    in_=rscratch[0].rearrange("(ic p) -> p ic", p=P),
                )
                recip_pp = rcp.tile([P, NT], f32, name="recip_pp")
                nc.vector.reciprocal(out=recip_pp[:], in_=den_pp[:])

                # ---- numerator (unnormalized, accumulate over j) -> numT [e, i] ----
                numT = ohp.tile([P, DC, N], bf16, name="numT")
                for ec in range(DC):
                    pn = [mmps.tile([P, FD], f32, name="mm") for _ in range(IH)]
                    for jc in range(NT):
                        for ih in range(IH):
                            nc.tensor.matmul(
                                pn[ih][:],
                                lhsT=vh[:, jc, ec * P:(ec + 1) * P],
                                rhs=et[:, jc, ih * FD:(ih + 1) * FD],
                                start=(jc == 0),
                                stop=(jc == NT - 1),
                            )
                    for ih in range(IH):
                        nc.scalar.copy(out=numT[:, ec, ih * FD:(ih + 1) * FD], in_=pn[ih][:])

                # ---- output projection; normalize per-row (i on partitions) and
                #      accumulate over heads in SBUF: rep = pr*recip[i] + rep ----
                for ic in range(NT):
                    # last head: borrow the idle mm-psum slots so all 8 rep
                    # groups can run before the recip chain lands
                    if h == H - 1 and ic >= NT - 4:
                        pr = mmps.tile([P, FD], f32, name="mm")
                    else:
                        pr = repps.tile([P, FD], f32, name="pr")
                    for ec in range(DC):
                        nc.tensor.matmul(
                            pr[:],
                            lhsT=numT[:, ec, ic * P:(ic + 1) * P],
                            rhs=wo[:, ec, :],
                            start=(ec == 0),
                            stop=(ec == DC - 1),
                        )
                    nc.vector.scalar_tensor_tensor(
                        out=rep[:, ic, :],
                        in0=pr[:],
                        scalar=recip_pp[:, ic:ic + 1],
                        in1=cv_full[:] if h == 0 else rep[:, ic, :],
                        op0=mybir.AluOpType.mult,
                        op1=mybir.AluOpType.add,
                    )
                    if h == H - 1:
                        nc.sync.dma_start(
                            out=out_d[b, ic * P:(ic + 1) * P, :], in_=rep[:, ic, :]
                        )

    nc.finalize()
    return nc


def _prep(k, v, q, Wk, bk, Wv, bv, Wq, bq, Wo, bo):
    """Host-side layout prep shared by all cores."""
    scale = np.float32(D ** -0.5)
    qT = np.ascontiguousarray(q.transpose(0, 2, 1)).astype(BF16)  # [B, D, N]
    kT = np.ascontiguousarray(k.transpose(0, 2, 1)).astype(BF16)
    vT = np.ascontiguousarray(v.transpose(0, 2, 1)).astype(BF16)
    WqT = np.ascontiguousarray(Wq.transpose(0, 2, 1)).astype(BF16)  # [H, d, e]
    WkT = np.ascontiguousarray(Wk.transpose(0, 2, 1)).astype(BF16)
    WvT = np.ascontiguousarray(Wv.transpose(0, 2, 1)).astype(BF16)
    WoR = Wo.reshape(D, D, H)  # [o, e, h]
    WoTh = np.ascontiguousarray(WoR.transpose(2, 1, 0)).astype(BF16)  # [h, e, o]
    bqp = np.ascontiguousarray(
        (bq * scale).reshape(H, DC, P).transpose(0, 2, 1)
    ).astype(np.float32)  # [H, P, DC]
    cv = (bo + np.einsum("oeh,he->o", WoR, bv)).astype(np.float32).reshape(1, D)
    return qT, kT, vT, WqT, WkT, WvT, WoTh, bqp, cv


def kernel(k, v, q, Wk, bk, Wv, bv, Wq, bq, Wo, bo):
    from concourse import bass_utils

    if "nc" not in _cached:
        _cached["nc"] = _build()
    nc = _cached["nc"]

    qT, kT, vT, WqT, WkT, WvT, WoTh, bqp, cv = _prep(
        k, v, q, Wk, bk, Wv, bv, Wq, bq, Wo, bo
    )

    in_maps = []
    for c in range(NCORES):
        sl = slice(c * BPC, (c + 1) * BPC)
        in_maps.append(
            {
                "qT": qT[sl],
                "kT": kT[sl],
                "vT": vT[sl],
                "WqT": WqT,
                "WkT": WkT,
                "WvT": WvT,
                "WoTh": WoTh,
                "bqp": bqp,
                "cv": cv,
            }
        )

    res = bass_utils.run_bass_kernel_spmd(nc, in_maps, core_ids=list(range(NCORES)))
    out = np.concatenate([r["out"] for r in res.results], axis=0)
    return out.astype(np.float32)


# revision 19
# speedup vs baseline: 1.1908x; 1.1908x over previous
"""Multi-head attention (per-head full-dim projections) on 8 TRN2 NeuronCores.

Problem: B=16, N=1024, D=512, H=8
  k_/v_/q_ = x @ W{k,v,q}[h].T + b  -> per-head [B,N,D]
  attn = softmax((q_ @ k_^T)/sqrt(D)); out = attn @ v_
  rep = interleave_heads(out) @ Wo.T + bo

Sharding: data parallel over batch (2 batches/core, no collectives).

Device kernel math notes (per core, per batch b, head h):
  - All activations kept "T-layout" [feature, token] so every matmul contracts
    over the partition dim with no on-device transposes:
      qhT[e,i] = WqT[h][d,e].T-mm  (lhsT=WqT tile, rhs=qT)  (+bq, x D^-0.5)
      khT[e,j] likewise (bias bk dropped: constant-per-row shift cancels in
      softmax), vh[j,e] = (lhsT=vT tile, rhs=WvT)  (bias bv folded into cv)
      S_T[j,i] = (lhsT=khT tile, rhs=qhT);  E_T = exp(S_T)  (no max-subtract:
      scores ~ N(0,1), max |s| ~ 6, exp is safe in fp32/bf16)
      den[1,i] = ones.T @ E_T;  recip = 1/den  (broadcast via DRAM roundtrip)
      numT -> oh[i,e]: actually num[i? no:] numT computed as
      rep[i,o] += (lhsT=ohT tile, rhs=WoTh[h]) accumulated over heads in SBUF
  - cv[o] = bo[o] + sum_h sum_e bv[h,e]*Wo[o, e*H+h] precomputed on host and
    added once (v-bias shifts oh by a constant vector -> constant rep shift).
"""

import sys

sys.path.insert(0, "/opt/trn_rl_repo")

from contextlib import ExitStack

import numpy as np
import ml_dtypes

B, N, D, H = 16, 1024, 512, 8
NCORES = 8
BPC = B // NCORES  # batches per core
P = 128
DC = D // P        # 4 feature chunks
NT = N // P        # 8 token chunks
FD = 512           # matmul moving free dim / PSUM bank
IH = N // FD       # 2 halves of the token axis

BF16 = ml_dtypes.bfloat16

_cached = {}


def _build():
    import concourse.bass as bass
    import concourse.tile as tile
    from concourse import bacc, mybir

    f32 = mybir.dt.float32
    bf16 = mybir.dt.bfloat16

    nc = bacc.Bacc(None, target_bir_lowering=False, debug=False)

    qT_d = nc.dram_tensor("qT", [BPC, D, N], bf16, kind="ExternalInput")
    kT_d = nc.dram_tensor("kT", [BPC, D, N], bf16, kind="ExternalInput")
    vT_d = nc.dram_tensor("vT", [BPC, D, N], bf16, kind="ExternalInput")
    wq_d = nc.dram_tensor("WqT", [H, D, D], bf16, kind="ExternalInput")
    wk_d = nc.dram_tensor("WkT", [H, D, D], bf16, kind="ExternalInput")
    wv_d = nc.dram_tensor("WvT", [H, D, D], bf16, kind="ExternalInput")
    wo_d = nc.dram_tensor("WoTh", [H, D, D], bf16, kind="ExternalInput")
    bq_d = nc.dram_tensor("bqp", [H, P, DC], f32, kind="ExternalInput")
    cv_d = nc.dram_tensor("cv", [1, D], f32, kind="ExternalInput")
    out_d = nc.dram_tensor("out", [BPC, N, D], f32, kind="ExternalOutput")

    scale = float(D) ** -0.5

    with tile.TileContext(nc) as tc, ExitStack() as ctx:
        consts = ctx.enter_context(tc.tile_pool(name="consts", bufs=1))
        acts = ctx.enter_context(tc.tile_pool(name="acts", bufs=1))
        wpool = ctx.enter_context(tc.tile_pool(name="wpool", bufs=2))
        projp = ctx.enter_context(tc.tile_pool(name="projp", bufs=2))
        etp = ctx.enter_context(tc.tile_pool(name="etp", bufs=1))
        ohp = ctx.enter_context(tc.tile_pool(name="ohp", bufs=2))
        rcp = ctx.enter_context(tc.tile_pool(name="rcp", bufs=2))
        repp = ctx.enter_context(tc.tile_pool(name="repp", bufs=2))
        mmps = ctx.enter_context(tc.tile_pool(name="mmps", bufs=4, space="PSUM"))
        repps = ctx.enter_context(tc.tile_pool(name="repps", bufs=4, space="PSUM"))
        dramp = ctx.enter_context(tc.tile_pool(name="dramp", bufs=2, space="DRAM"))

        bq_sb = consts.tile([P, H, DC], f32)
        nc.sync.dma_start(out=bq_sb[:], in_=bq_d[:].rearrange("h p c -> p h c"))
        cv_full = consts.tile([P, D], f32)
        nc.sync.dma_start(out=cv_full[:], in_=cv_d[0:1, :].to_broadcast([P, D]))

        for b in range(BPC):
            def load_weights(h):
                wq = wpool.tile([P, DC, D], bf16, name="wq")
                wk = wpool.tile([P, DC, D], bf16, name="wk")
                wv = wpool.tile([P, DC, D], bf16, name="wv")
                wo = wpool.tile([P, DC, D], bf16, name="wo")
                nc.sync.dma_start(out=wq[:], in_=wq_d[h].rearrange("(c p) e -> p c e", p=P))
                nc.sync.dma_start(out=wk[:], in_=wk_d[h].rearrange("(c p) e -> p c e", p=P))
                nc.sync.dma_start(out=wv[:], in_=wv_d[h].rearrange("(c p) e -> p c e", p=P))
                nc.sync.dma_start(out=wo[:], in_=wo_d[h].rearrange("(c p) e -> p c e", p=P))
                return wq, wk, wv, wo

            qT = acts.tile([P, DC, N], bf16, name="qT_sb")
            kT = acts.tile([P, DC, N], bf16, name="kT_sb")
            vT = acts.tile([P, DC, N], bf16, name="vT_sb")
            # interleave weight-h0 and activation chunk DMAs so the first
            # projection matmuls (wq dc0 + qT dc0) can start ASAP
            w0 = None
            for dc in range(DC):
                if b == 0 and dc == 0:
                    nc.sync.dma_start(out=qT[:, 0, :], in_=qT_d[b, 0:P, :])
                    w0 = load_weights(0)
                    nc.sync.dma_start(out=kT[:, 0, :], in_=kT_d[b, 0:P, :])
                    nc.sync.dma_start(out=vT[:, 0, :], in_=vT_d[b, 0:P, :])
                    continue
                nc.sync.dma_start(out=qT[:, dc, :], in_=qT_d[b, dc * P:(dc + 1) * P, :])
                nc.sync.dma_start(out=kT[:, dc, :], in_=kT_d[b, dc * P:(dc + 1) * P, :])
                nc.sync.dma_start(out=vT[:, dc, :], in_=vT_d[b, dc * P:(dc + 1) * P, :])

            rep = repp.tile([P, NT, D], f32, name="rep")

            for h in range(H):
                if b == 0 and h == 0:
                    wq, wk, wv, wo = w0
                else:
                    wq, wk, wv, wo = load_weights(h)

                qhT = projp.tile([P, DC, N], bf16, name="qhT")  # [e, i]
                khT = projp.tile([P, DC, N], bf16, name="khT")  # [e, j]
                vh = projp.tile([P, NT, D], bf16, name="vh")    # [j, e]

                # ---- projections ----
                for ec in range(DC):
                    pq = [mmps.tile([P, FD], f32, name="mm") for _ in range(IH)]
                    for dc in range(DC):
                        for ih in range(IH):
                            nc.tensor.matmul(
                                pq[ih][:],
                                lhsT=wq[:, dc, ec * P:(ec + 1) * P],
                                rhs=qT[:, dc, ih * FD:(ih + 1) * FD],
                                start=(dc == 0),
                                stop=(dc == DC - 1),
                            )
                    for ih in range(IH):
                        # qhT = psum*scale + bq*scale (bqp pre-scaled on host)
                        nc.scalar.activation(
                            out=qhT[:, ec, ih * FD:(ih + 1) * FD],
                            in_=pq[ih][:],
                            func=mybir.ActivationFunctionType.Identity,
                            bias=bq_sb[:, h, ec:ec + 1],
                            scale=scale,
                        )
                for ec in range(DC):
                    pk = [mmps.tile([P, FD], f32, name="mm") for _ in range(IH)]
                    for dc in range(DC):
                        for ih in range(IH):
                            nc.tensor.matmul(
                                pk[ih][:],
                                lhsT=wk[:, dc, ec * P:(ec + 1) * P],
                                rhs=kT[:, dc, ih * FD:(ih + 1) * FD],
                                start=(dc == 0),
                                stop=(dc == DC - 1),
                            )
                    for ih in range(IH):
                        nc.scalar.copy(out=khT[:, ec, ih * FD:(ih + 1) * FD], in_=pk[ih][:])
                for jc in range(NT):
                    pv = mmps.tile([P, FD], f32, name="mm")
                    for dc in range(DC):
                        nc.tensor.matmul(
                            pv[:],
                            lhsT=vT[:, dc, jc * P:(jc + 1) * P],
                            rhs=wv[:, dc, :],
                            start=(dc == 0),
                            stop=(dc == DC - 1),
                        )
                    nc.scalar.copy(out=vh[:, jc, :], in_=pv[:])

                # ---- scores + exp; den accumulated incrementally on DVE ----
                et = etp.tile([P, NT, N], bf16, name="et")  # E_T [j, i]
                den_acc = rcp.tile([P, N], f32, name="den_acc")
                for jc in range(NT):
                    ps = [mmps.tile([P, FD], f32, name="mm") for _ in range(IH)]
                    for ec in range(DC):
                        for ih in range(IH):
                            nc.tensor.matmul(
                                ps[ih][:],
                                lhsT=khT[:, ec, jc * P:(jc + 1) * P],
                                rhs=qhT[:, ec, ih * FD:(ih + 1) * FD],
                                start=(ec == 0),
                                stop=(ec == DC - 1),
                            )
                    for ih in range(IH):
                        nc.scalar.activation(
                            out=et[:, jc, ih * FD:(ih + 1) * FD],
                            in_=ps[ih][:],
                            func=mybir.ActivationFunctionType.Exp,
                        )
                    if jc == 1:
                        nc.vector.tensor_add(den_acc[:], et[:, 0, :], et[:, 1, :])
                    elif jc > 1:
                        nc.vector.tensor_add(den_acc[:], et[:, jc, :], den_acc[:])

                # ---- partition-reduce den on gpsimd, recip -> [i%P, i//P] ----
                from concourse import bass_isa

                nc.gpsimd.partition_all_reduce(
                    den_acc[:], den_acc[:], P, bass_isa.ReduceOp.add
                )
                rscratch = dramp.tile([1, N], f32, name="rscratch")
                nc.sync.dma_start(out=rscratch[:], in_=den_acc[0:1, :])
                den_pp = rcp.tile([P, NT], f32, name="den_pp")
                nc.sync.dma_start(
                    out=den_pp[:],
                    in_=rscratch[0].rearrange("(ic p) -> p ic", p=P),
                )
                recip_pp = rcp.tile([P, NT], f32, name="recip_pp")
                nc.vector.reciprocal(out=recip_pp[:], in_=den_pp[:])

                # ---- numerator (unnormalized, accumulate over j) -> numT [e, i] ----
                numT = ohp.tile([P, DC, N], bf16, name="numT")
                for ec in range(DC):
                    pn = [mmps.tile([P, FD], f32, name="mm") for _ in range(IH)]
                    for jc in range(NT):
                        for ih in range(IH):
                            nc.tensor.matmul(
                                pn[ih][:],
                                lhsT=vh[:, jc, ec * P:(ec + 1) * P],
                                rhs=et[:, jc, ih * FD:(ih + 1) * FD],
                                start=(jc == 0),
                                stop=(jc == NT - 1),
                            )
                    for ih in range(IH):
                        nc.scalar.copy(out=numT[:, ec, ih * FD:(ih + 1) * FD], in_=pn[ih][:])

                # ---- output projection; normalize per-row (i on partitions) and
                #      accumulate over heads in SBUF: rep = pr*recip[i] + rep ----
                for ic in range(NT):
                    pr = repps.tile([P, FD], f32, name="pr")
                    for ec in range(DC):
                        nc.tensor.matmul(
                            pr[:],
                            lhsT=numT[:, ec, ic * P:(ic + 1) * P],
                            rhs=wo[:, ec, :],
                            start=(ec == 0),
                            stop=(ec == DC - 1),
                        )
                    nc.vector.scalar_tensor_tensor(
                        out=rep[:, ic, :],
                        in0=pr[:],
                        scalar=recip_pp[:, ic:ic + 1],
                        in1=cv_full[:] if h == 0 else rep[:, ic, :],
                        op0=mybir.AluOpType.mult,
                        op1=mybir.AluOpType.add,
                    )
                    if h == H - 1:
                        nc.sync.dma_start(
                            out=out_d[b, ic * P:(ic + 1) * P, :], in_=rep[:, ic, :]
                        )

    nc.finalize()
    return nc


def _prep(k, v, q, Wk, bk, Wv, bv, Wq, bq, Wo, bo):
    """Host-side layout prep shared by all cores."""
    scale = np.float32(D ** -0.5)
    qT = np.ascontiguousarray(q.transpose(0, 2, 1)).astype(BF16)  # [B, D, N]
    kT = np.ascontiguousarray(k.transpose(0, 2, 1)).astype(BF16)
    vT = np.ascontiguousarray(v.transpose(0, 2, 1)).astype(BF16)
    WqT = np.ascontiguousarray(Wq.transpose(0, 2, 1)).astype(BF16)  # [H, d, e]
    WkT = np.ascontiguousarray(Wk.transpose(0, 2, 1)).astype(BF16)
    WvT = np.ascontiguousarray(Wv.transpose(0, 2, 1)).astype(BF16)
    WoR = Wo.reshape(D, D, H)  # [o, e, h]
    WoTh = np.ascontiguousarray(WoR.transpose(2, 1, 0)).astype(BF16)  # [h, e, o]
    bqp = np.ascontiguousarray(
        (bq * scale).reshape(H, DC, P).transpose(0, 2, 1)
    ).astype(np.float32)  # [H, P, DC]
    cv = (bo + np.einsum("oeh,he->o", WoR, bv)).astype(np.float32).reshape(1, D)
    return qT, kT, vT, WqT, WkT, WvT, WoTh, bqp, cv


def kernel(k, v, q, Wk, bk, Wv, bv, Wq, bq, Wo, bo):
    from concourse import bass_utils

    if "nc" not in _cached:
        _cached["nc"] = _build()
    nc = _cached["nc"]

    qT, kT, vT, WqT, WkT, WvT, WoTh, bqp, cv = _prep(
        k, v, q, Wk, bk, Wv, bv, Wq, bq, Wo, bo
    )

    in_maps = []
    for c in range(NCORES):
        sl = slice(c * BPC, (c + 1) * BPC)
        in_maps.append(
            {
                "qT": qT[sl],
                "kT": kT[sl],
                "vT": vT[sl],
                "WqT": WqT,
                "WkT": WkT,
                "WvT": WvT,
                "WoTh": WoTh,
                "bqp": bqp,
                "cv": cv,
            }
        )

    res = bass_utils.run_bass_kernel_spmd(nc, in_maps, core_ids=list(range(NCORES)))
    out = np.concatenate([r["out"] for r in res.results], axis=0)
    return out.astype(np.float32)


# revision 20
# speedup vs baseline: 1.1920x; 1.0010x over previous
"""Multi-head attention (per-head full-dim projections) on 8 TRN2 NeuronCores.

Problem: B=16, N=1024, D=512, H=8
  k_/v_/q_ = x @ W{k,v,q}[h].T + b  -> per-head [B,N,D]
  attn = softmax((q_ @ k_^T)/sqrt(D)); out = attn @ v_
  rep = interleave_heads(out) @ Wo.T + bo

Sharding: data parallel over batch (2 batches/core, no collectives).

Device kernel math notes (per core, per batch b, head h):
  - All activations kept "T-layout" [feature, token] so every matmul contracts
    over the partition dim with no on-device transposes:
      qhT[e,i] = WqT[h][d,e].T-mm  (lhsT=WqT tile, rhs=qT)  (+bq, x D^-0.5)
      khT[e,j] likewise (bias bk dropped: constant-per-row shift cancels in
      softmax), vh[j,e] = (lhsT=vT tile, rhs=WvT)  (bias bv folded into cv)
      S_T[j,i] = (lhsT=khT tile, rhs=qhT);  E_T = exp(S_T)  (no max-subtract:
      scores ~ N(0,1), max |s| ~ 6, exp is safe in fp32/bf16)
      den[1,i] = ones.T @ E_T;  recip = 1/den  (broadcast via DRAM roundtrip)
      numT -> oh[i,e]: actually num[i? no:] numT computed as
      rep[i,o] += (lhsT=ohT tile, rhs=WoTh[h]) accumulated over heads in SBUF
  - cv[o] = bo[o] + sum_h sum_e bv[h,e]*Wo[o, e*H+h] precomputed on host and
    added once (v-bias shifts oh by a constant vector -> constant rep shift).
"""

import sys

sys.path.insert(0, "/opt/trn_rl_repo")

from contextlib import ExitStack

import numpy as np
import ml_dtypes

B, N, D, H = 16, 1024, 512, 8
NCORES = 8
BPC = B // NCORES  # batches per core
P = 128
DC = D // P        # 4 feature chunks
NT = N // P        # 8 token chunks
FD = 512           # matmul moving free dim / PSUM bank
IH = N // FD       # 2 halves of the token axis

BF16 = ml_dtypes.bfloat16

_cached = {}


def _build():
    import concourse.bass as bass
    import concourse.tile as tile
    from concourse import bacc, mybir

    f32 = mybir.dt.float32
    bf16 = mybir.dt.bfloat16

    nc = bacc.Bacc(None, target_bir_lowering=False, debug=False)

    qT_d = nc.dram_tensor("qT", [BPC, P, DC, N], bf16, kind="ExternalInput")
    kT_d = nc.dram_tensor("kT", [BPC, P, DC, N], bf16, kind="ExternalInput")
    vT_d = nc.dram_tensor("vT", [BPC, P, DC, N], bf16, kind="ExternalInput")
    wq_d = nc.dram_tensor("WqT", [H, P, DC, D], bf16, kind="ExternalInput")
    wk_d = nc.dram_tensor("WkT", [H, P, DC, D], bf16, kind="ExternalInput")
    wv_d = nc.dram_tensor("WvT", [H, P, DC, D], bf16, kind="ExternalInput")
    wo_d = nc.dram_tensor("WoTh", [H, P, DC, D], bf16, kind="ExternalInput")
    bq_d = nc.dram_tensor("bqp", [H, P, DC], f32, kind="ExternalInput")
    cv_d = nc.dram_tensor("cv", [1, D], f32, kind="ExternalInput")
    out_d = nc.dram_tensor("out", [BPC, N, D], f32, kind="ExternalOutput")

    scale = float(D) ** -0.5

    with tile.TileContext(nc) as tc, ExitStack() as ctx:
        consts = ctx.enter_context(tc.tile_pool(name="consts", bufs=1))
        acts = ctx.enter_context(tc.tile_pool(name="acts", bufs=1))
        wpool = ctx.enter_context(tc.tile_pool(name="wpool", bufs=2))
        projp = ctx.enter_context(tc.tile_pool(name="projp", bufs=2))
        etp = ctx.enter_context(tc.tile_pool(name="etp", bufs=1))
        ohp = ctx.enter_context(tc.tile_pool(name="ohp", bufs=2))
        rcp = ctx.enter_context(tc.tile_pool(name="rcp", bufs=2))
        repp = ctx.enter_context(tc.tile_pool(name="repp", bufs=2))
        mmps = ctx.enter_context(tc.tile_pool(name="mmps", bufs=4, space="PSUM"))
        repps = ctx.enter_context(tc.tile_pool(name="repps", bufs=4, space="PSUM"))
        dramp = ctx.enter_context(tc.tile_pool(name="dramp", bufs=2, space="DRAM"))

        bq_sb = consts.tile([P, H, DC], f32)
        nc.gpsimd.dma_start(out=bq_sb[:], in_=bq_d[:].rearrange("h p c -> p h c"))
        cv_full = consts.tile([P, D], f32)
        nc.gpsimd.dma_start(out=cv_full[:], in_=cv_d[0:1, :].to_broadcast([P, D]))

        for b in range(BPC):
            def load_weights(h):
                wq = wpool.tile([P, DC, D], bf16, name="wq")
                wk = wpool.tile([P, DC, D], bf16, name="wk")
                wv = wpool.tile([P, DC, D], bf16, name="wv")
                wo = wpool.tile([P, DC, D], bf16, name="wo")
                nc.sync.dma_start(out=wq[:], in_=wq_d[h])
                nc.sync.dma_start(out=wk[:], in_=wk_d[h])
                nc.sync.dma_start(out=wv[:], in_=wv_d[h])
                nc.sync.dma_start(out=wo[:], in_=wo_d[h])
                return wq, wk, wv, wo

            qT = acts.tile([P, DC, N], bf16, name="qT_sb")
            kT = acts.tile([P, DC, N], bf16, name="kT_sb")
            vT = acts.tile([P, DC, N], bf16, name="vT_sb")
            # interleave weight-h0 and activation chunk DMAs so the first
            # projection matmuls (wq dc0 + qT dc0) can start ASAP
            w0 = None
            for dc in range(DC):
                if b == 0 and dc == 0:
                    nc.sync.dma_start(out=qT[:, 0, :], in_=qT_d[b, :, 0, :])
                    w0 = load_weights(0)
                    nc.sync.dma_start(out=kT[:, 0, :], in_=kT_d[b, :, 0, :])
                    nc.sync.dma_start(out=vT[:, 0, :], in_=vT_d[b, :, 0, :])
                    continue
                nc.sync.dma_start(out=qT[:, dc, :], in_=qT_d[b, :, dc, :])
                nc.sync.dma_start(out=kT[:, dc, :], in_=kT_d[b, :, dc, :])
                nc.sync.dma_start(out=vT[:, dc, :], in_=vT_d[b, :, dc, :])

            rep = repp.tile([P, NT, D], f32, name="rep")

            for h in range(H):
                if b == 0 and h == 0:
                    wq, wk, wv, wo = w0
                else:
                    wq, wk, wv, wo = load_weights(h)

                qhT = projp.tile([P, DC, N], bf16, name="qhT")  # [e, i]
                khT = projp.tile([P, DC, N], bf16, name="khT")  # [e, j]
                vh = projp.tile([P, NT, D], bf16, name="vh")    # [j, e]

                # ---- projections ----
                for ec in range(DC):
                    pq = [mmps.tile([P, FD], f32, name="mm") for _ in range(IH)]
                    for dc in range(DC):
                        for ih in range(IH):
                            nc.tensor.matmul(
                                pq[ih][:],
                                lhsT=wq[:, dc, ec * P:(ec + 1) * P],
                                rhs=qT[:, dc, ih * FD:(ih + 1) * FD],
                                start=(dc == 0),
                                stop=(dc == DC - 1),
                            )
                    for ih in range(IH):
                        # qhT = psum*scale + bq*scale (bqp pre-scaled on host)
                        nc.scalar.activation(
                            out=qhT[:, ec, ih * FD:(ih + 1) * FD],
                            in_=pq[ih][:],
                            func=mybir.ActivationFunctionType.Identity,
                            bias=bq_sb[:, h, ec:ec + 1],
                            scale=scale,
                        )
                for ec in range(DC):
                    pk = [mmps.tile([P, FD], f32, name="mm") for _ in range(IH)]
                    for dc in range(DC):
                        for ih in range(IH):
                            nc.tensor.matmul(
                                pk[ih][:],
                                lhsT=wk[:, dc, ec * P:(ec + 1) * P],
                                rhs=kT[:, dc, ih * FD:(ih + 1) * FD],
                                start=(dc == 0),
                                stop=(dc == DC - 1),
                            )
                    for ih in range(IH):
                        nc.scalar.copy(out=khT[:, ec, ih * FD:(ih + 1) * FD], in_=pk[ih][:])
                for jc in range(NT):
                    pv = mmps.tile([P, FD], f32, name="mm")
                    for dc in range(DC):
                        nc.tensor.matmul(
                            pv[:],
                            lhsT=vT[:, dc, jc * P:(jc + 1) * P],
                            rhs=wv[:, dc, :],
                            start=(dc == 0),
                            stop=(dc == DC - 1),
                        )
                    nc.scalar.copy(out=vh[:, jc, :], in_=pv[:])

                # ---- scores + exp; den accumulated incrementally on DVE ----
                et = etp.tile([P, NT, N], bf16, name="et")  # E_T [j, i]
                den_acc = rcp.tile([P, N], f32, name="den_acc")
                for jc in range(NT):
                    ps = [mmps.tile([P, FD], f32, name="mm") for _ in range(IH)]
                    for ec in range(DC):
                        for ih in range(IH):
                            nc.tensor.matmul(
                                ps[ih][:],
                                lhsT=khT[:, ec, jc * P:(jc + 1) * P],
                                rhs=qhT[:, ec, ih * FD:(ih + 1) * FD],
                                start=(ec == 0),
                                stop=(ec == DC - 1),
                            )
                    for ih in range(IH):
                        nc.scalar.activation(
                            out=et[:, jc, ih * FD:(ih + 1) * FD],
                            in_=ps[ih][:],
                            func=mybir.ActivationFunctionType.Exp,
                        )
                    if jc == 1:
                        nc.vector.tensor_add(den_acc[:], et[:, 0, :], et[:, 1, :])
                    elif jc > 1:
                        nc.vector.tensor_add(den_acc[:], et[:, jc, :], den_acc[:])

                # ---- partition-reduce den on gpsimd, recip -> [i%P, i//P] ----
                from concourse import bass_isa

                nc.gpsimd.partition_all_reduce(
                    den_acc[:], den_acc[:], P, bass_isa.ReduceOp.add
                )
                rscratch = dramp.tile([1, N], f32, name="rscratch")
                nc.gpsimd.dma_start(out=rscratch[:], in_=den_acc[0:1, :])
                den_pp = rcp.tile([P, NT], f32, name="den_pp")
                nc.gpsimd.dma_start(
                    out=den_pp[:],
                    in_=rscratch[0].rearrange("(ic p) -> p ic", p=P),
                )
                recip_pp = rcp.tile([P, NT], f32, name="recip_pp")
                nc.vector.reciprocal(out=recip_pp[:], in_=den_pp[:])

                # ---- numerator (unnormalized, accumulate over j) -> numT [e, i] ----
                numT = ohp.tile([P, DC, N], bf16, name="numT")
                for ec in range(DC):
                    pn = [mmps.tile([P, FD], f32, name="mm") for _ in range(IH)]
                    for jc in range(NT):
                        for ih in range(IH):
                            nc.tensor.matmul(
                                pn[ih][:],
                                lhsT=vh[:, jc, ec * P:(ec + 1) * P],
                                rhs=et[:, jc, ih * FD:(ih + 1) * FD],
                                start=(jc == 0),
                                stop=(jc == NT - 1),
                            )
                    for ih in range(IH):
                        nc.scalar.copy(out=numT[:, ec, ih * FD:(ih + 1) * FD], in_=pn[ih][:])

                # ---- output projection; normalize per-row (i on partitions) and
                #      accumulate over heads in SBUF: rep = pr*recip[i] + rep ----
                for ic in range(NT):
                    pr = repps.tile([P, FD], f32, name="pr")
                    for ec in range(DC):
                        nc.tensor.matmul(
                            pr[:],
                            lhsT=numT[:, ec, ic * P:(ic + 1) * P],
                            rhs=wo[:, ec, :],
                            start=(ec == 0),
                            stop=(ec == DC - 1),
                        )
                    nc.vector.scalar_tensor_tensor(
                        out=rep[:, ic, :],
                        in0=pr[:],
                        scalar=recip_pp[:, ic:ic + 1],
                        in1=cv_full[:] if h == 0 else rep[:, ic, :],
                        op0=mybir.AluOpType.mult,
                        op1=mybir.AluOpType.add,
                    )
                    if h == H - 1:
                        nc.sync.dma_start(
                            out=out_d[b, ic * P:(ic + 1) * P, :], in_=rep[:, ic, :]
                        )

    nc.finalize()
    return nc


def _prep(k, v, q, Wk, bk, Wv, bv, Wq, bq, Wo, bo):
    """Host-side layout prep shared by all cores."""
    scale = np.float32(D ** -0.5)
    def arr_x(x):  # [B?, D, N] -> [B?, P, DC, N]  (d = dc*P + p)
        b = x.shape[0]
        n = x.shape[2]
        return np.ascontiguousarray(
            x.reshape(b, DC, P, n).transpose(0, 2, 1, 3)
        ).astype(BF16)

    qT = arr_x(q.transpose(0, 2, 1))
    kT = arr_x(k.transpose(0, 2, 1))
    vT = arr_x(v.transpose(0, 2, 1))
    WqT = arr_x(Wq.transpose(0, 2, 1))  # [H, P, DC, e]
    WkT = arr_x(Wk.transpose(0, 2, 1))
    WvT = arr_x(Wv.transpose(0, 2, 1))
    WoR = Wo.reshape(D, D, H)  # [o, e, h]
    WoTh = arr_x(WoR.transpose(2, 1, 0))  # [h, P, DC, o]
    bqp = np.ascontiguousarray(
        (bq * scale).reshape(H, DC, P).transpose(0, 2, 1)
    ).astype(np.float32)  # [H, P, DC]
    cv = (bo + np.einsum("oeh,he->o", WoR, bv)).astype(np.float32).reshape(1, D)
    return qT, kT, vT, WqT, WkT, WvT, WoTh, bqp, cv


def kernel(k, v, q, Wk, bk, Wv, bv, Wq, bq, Wo, bo):
    from concourse import bass_utils

    if "nc" not in _cached:
        _cached["nc"] = _build()
    nc = _cached["nc"]

    qT, kT, vT, WqT, WkT, WvT, WoTh, bqp, cv = _prep(
        k, v, q, Wk, bk, Wv, bv, Wq, bq, Wo, bo
    )

    in_maps = []
    for c in range(NCORES):
        sl = slice(c * BPC, (c + 1) * BPC)
        in_maps.append(
            {
                "qT": qT[sl],
                "kT": kT[sl],
                "vT": vT[sl],
                "WqT": WqT,
                "WkT": WkT,
                "WvT": WvT,
                "WoTh": WoTh,
                "bqp": bqp,
                "cv": cv,
            }
        )

    res = bass_utils.run_bass_kernel_spmd(nc, in_maps, core_ids=list(range(NCORES)))
    out = np.concatenate([r["out"] for r in res.results], axis=0)
    return out.astype(np.float32)
